# revision 1
# baseline (speedup 1.0000x reference)
"""Multi-head attention (B=4, d_model=512, N=2048, H=8) on 8 Trainium2 cores.

Sharding: core c handles batch b = c//2 and head-group hg = c%2 (4 heads).
Each core computes its heads' q/k/v projections, attention, and a partial
output merge (Wm restricted to its heads' channels).  The host sums the two
partials per batch and adds the folded bias (bm + Wm @ bv).

Layout tricks:
  - Channels are permuted head-major on the host (weights are pre-sliced /
    pre-transposed per core), so each head occupies 64 contiguous partitions.
  - Scores are computed transposed (s_T[m, n] = k_h^T q_h) so that the
    softmax denominator falls out of the PV matmul via a ones-column
    appended to v_h^T; no max-subtraction is needed (|scores/8| <~ 6, exp is
    safe in fp32).
  - k-bias cancels in softmax; v-bias folds into the output bias (host).
  - All matmuls run as float32r (full PE rate for free-dim >= 256).
"""

import sys

for _p in ("/opt/trn_rl_repo",):
    if _p not in sys.path:
        sys.path.insert(0, _p)

from contextlib import ExitStack

import numpy as np

import concourse.bass as bass
import concourse.mybir as mybir
import concourse.tile as tile
from concourse import bacc
from concourse.bass_utils import run_bass_kernel_spmd

F32 = mybir.dt.float32
F32R = mybir.dt.float32r
EXP = mybir.ActivationFunctionType.Exp

B = 4
D = 512  # d_model
N_FULL = 2048
H = 8
HD = 64  # head dim
HPC = 4  # heads per core
C = HPC * HD  # 256 local channels per core
KT = D // 128  # contraction tiles for projections
CT = C // 128  # local channel tiles
NCORES = 8


def _mm(nc, out, lhsT, rhs, **kw):
    nc.tensor.matmul(
        out, lhsT.bitcast(F32R), rhs.bitcast(F32R), skip_group_check=True, **kw
    )


def build_program(N=N_FULL, debug=False, reps=1):
    """Build the single-core Bass program (SPMD across 8 cores)."""
    NHALF = N // 2
    MT = N // 128  # tiles of the m (key position) axis
    SW = min(512, NHALF)  # fp32 moving-operand slice width
    NHS = NHALF // SW  # slices per n-half
    NS = N // SW  # slices of full n

    nc = bacc.Bacc(
        "TRN2",
        target_bir_lowering=False,
        debug=False,
        enable_asserts=False,
        num_devices=NCORES,
    )

    xq_d = nc.declare_dram_parameter("xq", [D, N], F32R, isOutput=False).ap()
    xk_d = nc.declare_dram_parameter("xk", [D, N], F32R, isOutput=False).ap()
    xv_d = nc.declare_dram_parameter("xv", [D, N], F32R, isOutput=False).ap()
    wq_d = nc.declare_dram_parameter("wqT", [D, C], F32R, isOutput=False).ap()
    wk_d = nc.declare_dram_parameter("wkT", [D, C], F32R, isOutput=False).ap()
    wv_d = nc.declare_dram_parameter("wvT", [D, C], F32R, isOutput=False).ap()
    wm_d = nc.declare_dram_parameter("wmT", [C, D], F32R, isOutput=False).ap()
    bq_d = nc.declare_dram_parameter("bq2", [128, CT], F32, isOutput=False).ap()
    on_d = nc.declare_dram_parameter("onesc", [128, HPC], F32R, isOutput=False).ap()
    out_d = nc.declare_dram_parameter("out", [D, N], F32, isOutput=True).ap()
    if debug:
        dbg_q = nc.declare_dram_parameter("dbg_q", [128, N], F32, isOutput=True).ap()
        dbg_k = nc.declare_dram_parameter("dbg_k", [128, N], F32, isOutput=True).ap()
        dbg_vt = nc.declare_dram_parameter("dbg_vt", [128, HPC * (HD + 1)], F32, isOutput=True).ap()
        dbg_pr = nc.declare_dram_parameter("dbg_pr", [128, N // 2], F32, isOutput=True).ap()
        dbg_x = nc.declare_dram_parameter("dbg_x", [128, N], F32, isOutput=True).ap()
        dbg_rc = nc.declare_dram_parameter("dbg_rc", [1, N // 2], F32, isOutput=True).ap()
        dbg_bc = nc.declare_dram_parameter("dbg_bc", [64, N // 2], F32, isOutput=True).ap()
        dbg_den_sb = None

    with tile.TileContext(nc) as tc, ExitStack() as ctx:
        big = ctx.enter_context(tc.tile_pool(name="big", bufs=1))
        wp = ctx.enter_context(tc.tile_pool(name="wp", bufs=1))
        pk = ctx.enter_context(tc.tile_pool(name="pk", bufs=1))
        sm = ctx.enter_context(tc.tile_pool(name="sm", bufs=2))
        pp = ctx.enter_context(tc.tile_pool(name="pp", bufs=1, space="PSUM"))

        PTAGS = ["sA", "sB", "xA", "xB"]
        if debug:
            dbg_den_sb = sm.tile([1, N // 2], F32, tag="dbgden", bufs=1, name="dbgden")

        def emit_body(rep):
            xq_src = out_d.bitcast(F32R) if rep > 0 else xq_d
            # ---- ACT exp-table preload ------------------------------------
            warm = wp.tile([1, 16], F32, tag="warm", name="warm")
            nc.vector.memset(warm, 0.0)
            nc.scalar.activation(warm[0:1, 8:16], warm[0:1, 0:8], EXP, scale=1.0)

            # ---- loads: weights, then x tensors interleaved by column grp --
            wq_sb, wk_sb, wv_sb = [], [], []
            xq_sb, xk_sb, xv_sb = [], [], []
            for kt in range(KT):
                t = wp.tile([128, C], F32R, tag=f"wq{kt}", name=f"wq{kt}")
                nc.sync.dma_start(t, wq_d[kt * 128 : (kt + 1) * 128, :])
                wq_sb.append(t)
            bq_sb = wp.tile([128, CT], F32, tag="bq", name="bq")
            nc.sync.dma_start(bq_sb, bq_d)
            on_sb = wp.tile([128, HPC], F32R, tag="onesc", name="onesc")
            nc.sync.dma_start(on_sb, on_d)
            for kt in range(KT):
                t = wp.tile([128, C], F32R, tag=f"wk{kt}", name=f"wk{kt}")
                nc.sync.dma_start(t, wk_d[kt * 128 : (kt + 1) * 128, :])
                wk_sb.append(t)
            for kt in range(KT):
                t = wp.tile([128, C], F32R, tag=f"wv{kt}", name=f"wv{kt}")
                nc.sync.dma_start(t, wv_d[kt * 128 : (kt + 1) * 128, :])
                wv_sb.append(t)
            wm_sb = []
            for ct in range(CT):
                t = wp.tile([128, D], F32R, tag=f"wm{ct}", name=f"wm{ct}")
                nc.sync.dma_start(t, wm_d[ct * 128 : (ct + 1) * 128, :])
                wm_sb.append(t)
            for kt in range(KT):
                t = big.tile([128, N], F32R, tag=f"xq{kt}", name=f"xqt{kt}")
                xq_sb.append(t)
                t = big.tile([128, N], F32R, tag=f"xk{kt}", name=f"xkt{kt}")
                xk_sb.append(t)
                t = big.tile([128, N], F32R, tag=f"xv{kt}", name=f"xvt{kt}")
                xv_sb.append(t)

            def load_slices(dst, src_d, g):
                gs = slice(g * SW, (g + 1) * SW)
                for kt in range(KT):
                    nc.sync.dma_start(dst[kt][:, gs], src_d[kt * 128 : (kt + 1) * 128, gs])

            for g in range(min(2, NS)):
                load_slices(xq_sb, xq_d, g)
            load_slices(xk_sb, xk_d, 0)
            load_slices(xv_sb, xv_d, 0)
            for g in range(1, NS):
                load_slices(xk_sb, xk_d, g)
                load_slices(xv_sb, xv_d, g)
            for g in range(2, NS):
                load_slices(xq_sb, xq_d, g)

            # ---- projection emitters (psum tag "mg", 2 bufs) --------------
            q_sb, k_sb = [], []
            for ct in range(CT):
                t = pk.tile([128, N], F32R, tag=f"q{ct}", name=f"q{ct}")
                q_sb.append(t)
                t = pk.tile([128, N], F32R, tag=f"k{ct}", name=f"k{ct}")
                k_sb.append(t)
            vt_sb = []
            for mt in range(MT):
                t = pk.tile([128, HPC * (HD + 1)], F32R, tag=f"vt{mt}", name=f"vt{mt}")
                vt_sb.append(t)

            def q_proj(j, on_act):
                js = slice(j * SW, (j + 1) * SW)
                for ct in range(CT):
                    ps = pp.tile([128, SW], F32, tag="mg", bufs=2, name=f"qps{ct}_{j}")
                    for kt in range(KT):
                        _mm(
                            nc,
                            ps,
                            wq_sb[kt][:, ct * 128 : (ct + 1) * 128],
                            xq_sb[kt][:, js],
                            start=(kt == 0),
                            stop=(kt == KT - 1),
                        )
                    if on_act:
                        nc.scalar.add(q_sb[ct][:, js], ps, bq_sb[:, ct : ct + 1])
                    else:
                        nc.vector.tensor_scalar_add(
                            q_sb[ct][:, js], ps, bq_sb[:, ct : ct + 1]
                        )

            def k_proj(j, on_act):
                js = slice(j * SW, (j + 1) * SW)
                for ct in range(CT):
                    ps = pp.tile([128, SW], F32, tag="mg", bufs=2, name=f"kps{ct}_{j}")
                    for kt in range(KT):
                        _mm(
                            nc,
                            ps,
                            wk_sb[kt][:, ct * 128 : (ct + 1) * 128],
                            xk_sb[kt][:, js],
                            start=(kt == 0),
                            stop=(kt == KT - 1),
                        )
                    if on_act:
                        nc.scalar.copy(k_sb[ct][:, js], ps)
                    else:
                        nc.vector.tensor_copy(k_sb[ct][:, js], ps)

            def vt_proj(mt):
                t3 = vt_sb[mt].rearrange("p (h x) -> p h x", h=HPC)
                ps = pp.tile([128, C], F32, tag="mg", bufs=2, name=f"vps{mt}")
                for kt in range(KT):
                    _mm(
                        nc,
                        ps,
                        xv_sb[kt][:, mt * 128 : (mt + 1) * 128],
                        wv_sb[kt],
                        start=(kt == 0),
                        stop=(kt == KT - 1),
                    )
                nc.vector.tensor_copy(
                    t3[:, :, 0:HD], ps.rearrange("p (h d) -> p h d", h=HPC)
                )
                nc.vector.tensor_copy(
                    t3[:, :, HD : HD + 1], on_sb.rearrange("p (h o) -> p h o", o=1)
                )

            # ---- attention: one head per block, software-pipelined m loop --
            x_sb = []
            for ct in range(CT):
                t = big.tile([128, N], F32R, tag=f"xq{ct}", name=f"x{ct}")
                x_sb.append(t)

            merge_groups = []  # deferred merge units (ot, nh) emitted inside blocks

            def merge_unit(ot, nh, act_copy):
                nbase = nh * NHALF
                ob = big.tile([128, NHALF], F32, tag=f"xq{2 + ot % 2}", name=f"ob{ot}{nh}")
                for j in range(NHS):
                    js = slice(nbase + j * SW, nbase + (j + 1) * SW)
                    ps = pp.tile([128, SW], F32, tag="mg", bufs=2, name=f"ops{ot}{nh}{j}")
                    for ct in range(CT):
                        _mm(
                            nc,
                            ps,
                            wm_sb[ct][:, ot * 128 : (ot + 1) * 128],
                            x_sb[ct][:, js],
                            start=(ct == 0),
                            stop=(ct == CT - 1),
                        )
                    ljs = slice(j * SW, (j + 1) * SW)
                    if act_copy:
                        nc.scalar.copy(ob[:, ljs], ps)
                    else:
                        nc.vector.tensor_copy(ob[:, ljs], ps)
                nc.sync.dma_start(
                    out_d[ot * 128 : (ot + 1) * 128, nbase : nbase + NHALF], ob
                )

            def attention_block(h, nh, ptags, hooks=None):
                hp, base = h // 2, (h % 2) * 64
                nbase = nh * NHALF

                def qk(s_tile, mt):
                    ms = slice(mt * 128, (mt + 1) * 128)
                    for j in range(NHS):
                        js = slice(j * SW, (j + 1) * SW)
                        gjs = slice(nbase + j * SW, nbase + (j + 1) * SW)
                        _mm(
                            nc,
                            s_tile[:, js],
                            k_sb[hp][base : base + 64, ms],
                            q_sb[hp][base : base + 64, gjs],
                            start=True,
                            stop=True,
                            tile_position=(base, 0),
                        )

                s_tiles = {}
                s_tiles[0] = pp.tile(
                    [128, NHALF], F32, tag="s", bufs=2, name=f"s{h}{nh}0"
                )
                qk(s_tiles[0], 0)
                x_ps = pp.tile([HD + 1, NHALF], F32, tag="x", name=f"xp{h}{nh}")
                if MT > 1:
                    s_tiles[1] = pp.tile(
                        [128, NHALF], F32, tag="s", bufs=2, name=f"s{h}{nh}1"
                    )
                    qk(s_tiles[1], 1)
                if hooks and -1 in hooks:
                    for fn in hooks[-1]:
                        fn()
                for mt in range(MT):
                    pr = big.tile(
                        [128, NHALF], F32R, tag=ptags[mt % len(ptags)],
                        name=f"pr{h}{nh}{mt}",
                    )
                    nc.scalar.activation(pr, s_tiles.pop(mt), EXP, scale=0.125)
                    if mt + 2 < MT:
                        s_tiles[mt + 2] = pp.tile(
                            [128, NHALF], F32, tag="s", bufs=2, name=f"s{h}{nh}{mt + 2}"
                        )
                        qk(s_tiles[mt + 2], mt + 2)
                    for j in range(NHS):
                        js = slice(j * SW, (j + 1) * SW)
                        _mm(
                            nc,
                            x_ps[:, js],
                            vt_sb[mt][:, h * (HD + 1) : (h + 1) * (HD + 1)],
                            pr[:, js],
                            start=(mt == 0),
                            stop=(mt == MT - 1),
                        )
                    if hooks and mt in hooks:
                        for fn in hooks[mt]:
                            fn()
                    elif hooks is None and merge_groups and mt in (5, 11):
                        ot, mnh = merge_groups.pop(0)
                        merge_unit(ot, mnh, act_copy=False)
                # normalize: x[d, n] / denom[n]  (denom in row HD)
                den_sb = sm.tile([1, NHALF], F32, tag="den", name=f"dn{h}{nh}")
                nc.vector.tensor_copy(den_sb, x_ps[HD : HD + 1, :])
                recip = sm.tile([1, NHALF], F32, tag="rec", name=f"rc{h}{nh}")
                nc.vector.reciprocal_approx_fast(out=recip, in_=den_sb)
                bc = sm.tile([64, NHALF], F32, tag="bc", bufs=2, name=f"bc{h}{nh}")
                nc.gpsimd.partition_broadcast(bc, recip)
                rows = slice((h % 2) * 64, (h % 2) * 64 + 64)
                nc.vector.tensor_mul(
                    x_sb[h // 2][rows, nbase : nbase + NHALF],
                    x_ps[0:HD, :],
                    bc,
                )

            # prelude: just enough projection for block 0 to start
            q_proj(0, on_act=True)
            q_proj(1, on_act=True)
            k_proj(0, on_act=True)
            for mt in range(4):
                vt_proj(mt)

            from collections import defaultdict
            from functools import partial

            b0_hooks = defaultdict(list)
            b0_hooks[-1].append(partial(k_proj, 1, on_act=False))
            for j in range(2, NS):
                b0_hooks[max(0, 4 * j - 6)].append(partial(k_proj, j, on_act=False))
            for m in range(4, MT):
                b0_hooks[m - 2].append(partial(vt_proj, m))
            for i, j in enumerate(range(2, NS)):
                b0_hooks[min(MT - 2 + i, MT - 1)].append(partial(q_proj, j, on_act=False))

            PTAGS_FIRST = ["xv0", "xv1", "xv2", "xv3"]
            PTAGS_REST = ["xv0", "xv1", "xv2", "xv3", "xk0", "xk1"]
            for nh in range(2):
                for h in range(HPC):
                    first = nh == 0 and h == 0
                    attention_block(
                        h, nh,
                        PTAGS_FIRST if first else PTAGS_REST,
                        hooks=b0_hooks if first else None,
                    )
                # queue this half's merge; it gets emitted inside later blocks
                merge_groups.extend([(ot, nh) for ot in range(KT)])
            # tail: whatever merge work wasn't absorbed into blocks
            for i, (ot, mnh) in enumerate(merge_groups):
                merge_unit(ot, mnh, act_copy=(i % 2 == 0))
            merge_groups.clear()

        for rep in range(reps):
            emit_body(rep)

        if debug:
            nc.sync.dma_start(dbg_q, q_sb[0].bitcast(F32))
            nc.sync.dma_start(dbg_k, k_sb[0].bitcast(F32))
            nc.sync.dma_start(dbg_vt, vt_sb[0].bitcast(F32))
            nc.sync.dma_start(dbg_x, x_sb[0].bitcast(F32))

    nc.compile()
    return nc


def make_in_maps(query, key, value, Wq, bq, Wk, Wv, Wm, n_cores=NCORES):
    query = np.asarray(query, np.float32)
    key = np.asarray(key, np.float32)
    value = np.asarray(value, np.float32)
    Wq = np.asarray(Wq, np.float32)
    bq = np.asarray(bq, np.float32)
    Wk = np.asarray(Wk, np.float32)
    Wv = np.asarray(Wv, np.float32)
    Wm = np.asarray(Wm, np.float32)
    in_maps = []
    for c in range(n_cores):
        b, hg = c // 2, c % 2
        heads = [hg * HPC + i for i in range(HPC)]
        mych = np.array([d * H + h for h in heads for d in range(HD)])
        in_maps.append(
            {
                "xq": np.ascontiguousarray(query[b]),
                "xk": np.ascontiguousarray(key[b]),
                "xv": np.ascontiguousarray(value[b]),
                "wqT": np.ascontiguousarray(Wq[mych].T),
                "wkT": np.ascontiguousarray(Wk[mych].T),
                "wvT": np.ascontiguousarray(Wv[mych].T),
                "wmT": np.ascontiguousarray(Wm[:, mych].T),
                "bq2": np.ascontiguousarray(bq[mych].reshape(CT, 128).T),
                "onesc": np.ones((128, HPC), np.float32),
            }
        )
    return in_maps


_PROG = {}


def _get_program(N=N_FULL):
    if N not in _PROG:
        _PROG[N] = build_program(N)
    return _PROG[N]


def kernel(query, key, value, Wq, bq, Wk, bk, Wv, bv, Wm, bm):
    nc = _get_program()
    in_maps = make_in_maps(query, key, value, Wq, bq, Wk, Wv, Wm)
    res = run_bass_kernel_spmd(nc, in_maps, list(range(NCORES))).results
    bm_eff = (np.asarray(Wm, np.float64) @ np.asarray(bv, np.float64)).astype(
        np.float32
    ) + np.asarray(bm, np.float32)
    out = np.empty((B, D, N_FULL), np.float32)
    for b in range(B):
        out[b] = res[2 * b]["out"] + res[2 * b + 1]["out"] + bm_eff[:, None]
    return out



# revision 50
# speedup vs baseline: 4.7060x; 4.7060x over previous
"""Multi-head attention (B=4, d_model=512, N=2048, H=8) on 8 Trainium2 cores.

Sharding: core c handles batch b = c//2 and head-group hg = c%2 (4 heads).
Each core computes its heads' q/k/v projections, attention, and a partial
output merge (Wm restricted to its heads' channels).  The host sums the two
partials per batch and adds the folded bias (bm + Wm @ bv).

v2 design (paired-head flat pipeline, all-bf16 matmuls):
  - All matmul operands are bf16 (1 cyc/col at any width, halves DMA+SBUF);
    accumulation stays fp32 in PSUM.  exp runs in fp32 out of PSUM.
  - ACT engine does exp ONLY (it is the roofline: 128 x [128,1024] exps
    ~= 118us); every copy/bias lands on DVE, broadcasts on Pool.
  - Heads are processed in pairs (2hp, 2hp+1), which live in rows 0:64 /
    64:128 of one SBUF tile.  The two QK matmuls of a pair use disjoint
    64-row PE groups (tile_position (0,0) / (64,0)) and are issued
    back-to-back so the PE array runs them concurrently (row tiling).
  - Flat global pipeline over 64 pair-steps (4 pair-blocks x 16 m-tiles):
    QK(step+1) is emitted before PV(step), so score tiles for the next
    step (and the next block) are always in flight; the exp stream never
    waits at block boundaries.
  - PSUM: score ring 2 x [128,1024] (4 banks) + per-lane x accumulators
    2 x [65,1024] (4 banks).  Projection/merge psums borrow score-ring
    slots ([:, :512] sub-APs).  x_ps is freed early by a DVE staging copy;
    normalization (recip + partition_broadcast + mul) runs off-band.
  - k-bias cancels in softmax; v-bias folds into the output bias (host).
"""

import sys

for _p in ("/opt/trn_rl_repo",):
    if _p not in sys.path:
        sys.path.insert(0, _p)

from collections import defaultdict
from contextlib import ExitStack
from functools import partial

import numpy as np

import concourse.bass as bass
import concourse.mybir as mybir
import concourse.tile as tile
from concourse import bacc
from concourse.bass_utils import run_bass_kernel_spmd

F32 = mybir.dt.float32
BF16 = mybir.dt.bfloat16
EXP = mybir.ActivationFunctionType.Exp

B = 4
D = 512  # d_model
N_FULL = 2048
H = 8
HD = 64  # head dim
HPC = 4  # heads per core
C = HPC * HD  # 256 local channels per core
KT = D // 128  # contraction tiles for projections
CT = C // 128  # local channel tiles (= head pairs)
NCORES = 8


def _mm(nc, out, lhsT, rhs, **kw):
    nc.tensor.matmul(out, lhsT, rhs, skip_group_check=True, **kw)


def build_program(N=N_FULL, reps=1, paired=True, debug=False):
    """Build the single-core Bass program (SPMD across 8 cores)."""
    NHALF = N // 2
    MT = N // 128  # tiles of the m (key position) axis
    SW = 512  # psum-bank slice width (fp32)
    NHS = NHALF // SW  # slices per n-half
    NS = N // SW  # slices of full n
    NPB = 2 * CT  # pair-blocks: (nh, hp)
    GTOT = NPB * MT  # global pair-steps

    nc = bacc.Bacc(
        "TRN2",
        target_bir_lowering=False,
        debug=False,
        enable_asserts=False,
        num_devices=NCORES,
    )

    xq_d = nc.declare_dram_parameter("xq", [D, N], BF16, isOutput=False).ap()
    xk_d = nc.declare_dram_parameter("xk", [D, N], BF16, isOutput=False).ap()
    xv_d = nc.declare_dram_parameter("xv", [D, N], BF16, isOutput=False).ap()
    wq_d = nc.declare_dram_parameter("wqT", [D, C], BF16, isOutput=False).ap()
    wk_d = nc.declare_dram_parameter("wkT", [D, C], BF16, isOutput=False).ap()
    wv_d = nc.declare_dram_parameter("wvT", [D, C], BF16, isOutput=False).ap()
    wm_d = nc.declare_dram_parameter("wmT", [C, D], BF16, isOutput=False).ap()
    bq_d = nc.declare_dram_parameter("bq2", [128, CT], F32, isOutput=False).ap()
    on_d = nc.declare_dram_parameter("onesc", [128, HPC], BF16, isOutput=False).ap()
    out_d = nc.declare_dram_parameter("out", [D, N], F32, isOutput=True).ap()
    if debug:
        dbg = {
            nm: nc.declare_dram_parameter(nm, shp, mybir.dt.bfloat16, isOutput=True).ap()
            for nm, shp in [
                ("dbg_q0", [128, N]), ("dbg_k0", [128, N]),
                ("dbg_vt0", [128, HPC * (HD + 1)]),
                ("dbg_x0", [128, N]), ("dbg_x1", [128, N]),
            ]
        }
        dbg["dbg_xp"] = nc.declare_dram_parameter("dbg_xp", [HD, N // 2], F32, isOutput=True).ap()
        dbg["dbg_rec"] = nc.declare_dram_parameter("dbg_rec", [1, N // 2], F32, isOutput=True).ap()

    with tile.TileContext(nc) as tc, ExitStack() as ctx:
        big = ctx.enter_context(tc.tile_pool(name="big", bufs=1))
        wp = ctx.enter_context(tc.tile_pool(name="wp", bufs=1))
        pk = ctx.enter_context(tc.tile_pool(name="pk", bufs=1))
        sm = ctx.enter_context(tc.tile_pool(name="sm", bufs=2))
        pp = ctx.enter_context(tc.tile_pool(name="pp", bufs=1, space="PSUM"))

        def emit_body(rep):
            xq_src = xq_d

            # ---- ACT exp-table preload ------------------------------------
            # rep>0 (timing chains): serialize on the previous rep's output
            # via a tiny scratch read into the warm tile; the memset then
            # overwrites it, so numerics are identical across reps while the
            # whole ACT stream (and everything downstream) gates on rep-1.
            warm = wp.tile([1, 16], F32, tag="warm", name="warm")
            if rep > 0:
                nc.sync.dma_start(warm, out_d[0:1, 0:16])
            nc.vector.memset(warm, 0.0)
            nc.scalar.activation(warm[0:1, 8:16], warm[0:1, 0:8], EXP, scale=1.0)

            # ---- SBUF tiles ----------------------------------------------
            # x / weight tensors hold all KT row-tiles in one tile so a
            # slice-group loads with a single (3D-AP) DMA.
            wq_sb = wp.tile([128, KT, C], BF16, tag="wq", name="wq")
            wk_sb = wp.tile([128, KT, C], BF16, tag="wk", name="wk")
            wv_sb = wp.tile([128, KT, C], BF16, tag="wv", name="wv")
            wm_sb = wp.tile([128, CT, D], BF16, tag="wm", name="wm")
            bq_sb = wp.tile([128, CT], F32, tag="bq", name="bq")
            on_sb = wp.tile([128, HPC], BF16, tag="onesc", name="onesc")
            on64f = wp.tile([1, 64], F32, tag="on64f", name="on64f")
            nc.vector.memset(on64f, 1.0)
            on64 = wp.tile([1, 64], mybir.dt.float32r, tag="on64", name="on64")
            nc.scalar.copy(on64, on64f)
            xq_sb = big.tile([128, KT, N], BF16, tag="xq", name="xq")
            xk_sb = big.tile([128, KT, N], BF16, tag="xk", name="xk")
            xv_sb = big.tile([128, KT, N], BF16, tag="xv", name="xv")
            q_sb = [pk.tile([128, N], BF16, tag=f"q{ct}", name=f"q{ct}") for ct in range(CT)]
            k_sb = [pk.tile([128, N], BF16, tag=f"k{ct}", name=f"k{ct}") for ct in range(CT)]
            vt_sb = [
                pk.tile([128, HPC * (HD + 1)], BF16, tag=f"vt{mt}", name=f"vt{mt}")
                for mt in range(MT)
            ]
            x_sb = [pk.tile([128, N], BF16, tag=f"x{ct}", name=f"x{ct}") for ct in range(CT)]

            # ---- DMA emission in consumption order ------------------------
            def dma_w(dst, src):  # [D, C] -> [128, KT, C], one transfer
                nc.sync.dma_start(
                    dst, src.rearrange("(kt p) c -> p kt c", p=128)
                )

            def dma_x(dst, src_d, g):  # one 512-col slice group, one transfer
                gs = slice(g * SW, (g + 1) * SW)
                nc.sync.dma_start(
                    dst[:, :, gs],
                    src_d.rearrange("(kt p) n -> p kt n", p=128)[:, :, gs],
                )

            dma_w(wq_sb, wq_d)
            dma_x(xq_sb, xq_src, 0)
            dma_w(wk_sb, wk_d)
            dma_x(xk_sb, xk_d, 0)
            dma_x(xq_sb, xq_src, 1)
            nc.sync.dma_start(bq_sb, bq_d)
            nc.sync.dma_start(on_sb, on_d)
            dma_x(xv_sb, xv_d, 0)
            dma_w(wv_sb, wv_d)
            dma_x(xk_sb, xk_d, 1)
            dma_x(xv_sb, xv_d, 1)
            dma_x(xk_sb, xk_d, 2)
            dma_x(xv_sb, xv_d, 2)
            dma_x(xk_sb, xk_d, 3)
            dma_x(xv_sb, xv_d, 3)
            dma_x(xq_sb, xq_src, 2)
            dma_x(xq_sb, xq_src, 3)
            nc.sync.dma_start(
                wm_sb, wm_d.rearrange("(ct p) d -> p ct d", p=128)
            )

            # ---- psum-ring helper (score ring shared with proj/merge) -----
            def sring(name):
                return pp.tile([128, NHALF], F32, tag="s", bufs=2, name=name)

            # ---- projection units -----------------------------------------
            def q_proj(j, ct):
                js = slice(j * SW, (j + 1) * SW)
                ps = sring(f"qps{j}_{ct}")
                for kt in range(KT):
                    _mm(
                        nc, ps[:, 0:SW],
                        wq_sb[:, kt, ct * 128 : (ct + 1) * 128],
                        xq_sb[:, kt, js],
                        start=(kt == 0), stop=(kt == KT - 1),
                    )
                nc.vector.tensor_scalar_add(q_sb[ct][:, js], ps[:, 0:SW], bq_sb[:, ct : ct + 1])

            def k_proj(j, ct, c0=0, c1=SW):
                js = slice(j * SW + c0, j * SW + c1)
                ps = sring(f"kps{j}_{ct}_{c0}")
                w = c1 - c0
                for kt in range(KT):
                    _mm(
                        nc, ps[:, 0:w],
                        wk_sb[:, kt, ct * 128 : (ct + 1) * 128],
                        xk_sb[:, kt, js],
                        start=(kt == 0), stop=(kt == KT - 1),
                    )
                nc.vector.tensor_copy(k_sb[ct][:, js], ps[:, 0:w])

            def vt_proj(mt):
                t3 = vt_sb[mt].rearrange("p (h x) -> p h x", h=HPC)
                ps = sring(f"vps{mt}")
                for kt in range(KT):
                    _mm(
                        nc, ps[:, 0:C],
                        xv_sb[:, kt, mt * 128 : (mt + 1) * 128],
                        wv_sb[:, kt, :],
                        start=(kt == 0), stop=(kt == KT - 1),
                    )
                nc.vector.tensor_copy(
                    t3[:, :, 0:HD], ps[:, 0:C].rearrange("p (h d) -> p h d", h=HPC)
                )
                nc.vector.tensor_copy(
                    t3[:, :, HD : HD + 1], on_sb.rearrange("p (h o) -> p h o", o=1)
                )

            # ---- merge units ----------------------------------------------
            ob_of = {}

            def merge_unit(ot, nh, j, act_copy=False):
                gjs = slice(nh * NHALF + j * SW, nh * NHALF + (j + 1) * SW)
                ps = sring(f"ops{ot}{nh}{j}")
                for ct in range(CT):
                    _mm(
                        nc, ps[:, 0:SW],
                        wm_sb[:, ct, ot * 128 : (ot + 1) * 128],
                        x_sb[ct][:, gjs],
                        start=(ct == 0), stop=(ct == CT - 1),
                    )
                ob = big.tile([128, SW], F32, tag=f"ob{(2 * ot + j) % 4}", bufs=2, name=f"ob{ot}{nh}{j}")
                if act_copy:
                    nc.scalar.copy(ob, ps[:, 0:SW])
                else:
                    nc.vector.tensor_copy(ob, ps[:, 0:SW])
                nc.sync.dma_start(out_d[ot * 128 : (ot + 1) * 128, gjs], ob)

            # ---- attention: flat pipeline over pair-steps -----------------
            # pair-blocks: (nh, hp); heads (2hp, 2hp+1) in rows 0:64/64:128
            PBS = [(0, 0), (0, 1), (1, 0), (1, 1)]
            sdict = {}
            xps = {}
            dbg_xt, dbg_rec = [], []

            def qk(g, lane):
                pb, mt = divmod(g, MT)
                nh, hp = PBS[pb]
                nbase = nh * NHALF
                base = lane * 64
                ms = slice(mt * 128, (mt + 1) * 128)
                t = sring(f"s{g}_{lane}")
                for j in range(NHS):
                    js = slice(j * SW, (j + 1) * SW)
                    gjs = slice(nbase + j * SW, nbase + (j + 1) * SW)
                    _mm(
                        nc, t[:, js],
                        k_sb[hp][base : base + 64, ms],
                        q_sb[hp][base : base + 64, gjs],
                        start=True, stop=True,
                        tile_position=(base, 0),
                    )
                sdict[(g, lane)] = t

            def pv(g, lane, pr):
                pb, mt = divmod(g, MT)
                nh, hp = PBS[pb]
                h = 2 * hp + lane
                if mt == 0:
                    xps[(pb, lane)] = pp.tile(
                        [HD + 1, NHALF], F32, tag=f"xa{lane}", bufs=1, name=f"xp{pb}{lane}"
                    )
                xp = xps[(pb, lane)]
                for j in range(NHS):
                    js = slice(j * SW, (j + 1) * SW)
                    _mm(
                        nc, xp[:, js],
                        vt_sb[mt][:, h * (HD + 1) : (h + 1) * (HD + 1)],
                        pr[:, js],
                        start=(mt == 0), stop=(mt == MT - 1),
                    )

            # normalize in three emission phases so DVE's FIFO never parks
            # behind a not-yet-satisfied dependency:
            #   a) psum->sbuf staging copy (frees x_ps) + reciprocal (bf16)
            #   b) PE ones-matmul broadcast of the reciprocal into a ring slot
            #   c) DVE multiply into x_sb
            def tail_a1(pb, lane, xp):
                # reciprocal_approx_fast must read SBUF: stage the den row
                # (partition-shift copy) first.
                den = sm.tile([1, NHALF], F32, tag=f"den{lane}", bufs=1, name=f"dn{pb}{lane}")
                nc.vector.tensor_copy(den, xp[HD : HD + 1, :])
                rec = sm.tile([1, NHALF], F32, tag=f"rec{lane}", bufs=1, name=f"rc{pb}{lane}")
                nc.vector.reciprocal_approx_fast(out=rec, in_=den)
                return rec

            def tail_a2(pb, lane, xp):
                xt = sm.tile([HD, NHALF], F32, tag=f"xt{lane}", bufs=1, name=f"xt{pb}{lane}")
                nc.vector.tensor_copy(xt, xp[0:HD, :])
                return xt

            F32R = mybir.dt.float32r

            def tail_b_pool(rec, pb, lane):
                bc = sm.tile([64, NHALF], F32, tag=f"bc{lane}", bufs=1, name=f"bc{pb}{lane}")
                nc.gpsimd.partition_broadcast(bc, rec)
                return bc

            def tail_b_pe(rec, pb, lane):
                # final block: no QK follows, so a score-ring slot is free for
                # a cheap PE ones-matmul broadcast instead of Pool.
                bc = sring(f"bcp{pb}{lane}")
                for j in range(NHS):
                    js = slice(j * SW, (j + 1) * SW)
                    _mm(
                        nc, bc[0:64, js], on64, rec[:, js],
                        start=True, stop=True,
                    )
                return bc

            def tail_c(xt, bc, pb, lane, c0=0, c1=NHALF):
                nh, hp = PBS[pb]
                nbase = nh * NHALF
                rows = slice(lane * 64, lane * 64 + 64)
                nc.vector.tensor_mul(
                    x_sb[hp][rows, nbase + c0 : nbase + c1],
                    xt[:, c0:c1],
                    bc[0:64, c0:c1],
                )

            def block_tail(pb):
                last = pb == NPB - 1
                if not last:
                    xp0, xp1 = xps.pop((pb, 0)), xps.pop((pb, 1))
                    rec0 = tail_a1(pb, 0, xp0)
                    rec1 = tail_a1(pb, 1, xp1)
                    bc0 = tail_b_pool(rec0, pb, 0)
                    bc1 = tail_b_pool(rec1, pb, 1)
                    xt0 = tail_a2(pb, 0, xp0)
                    xt1 = tail_a2(pb, 1, xp1)
                    tail_c(xt0, bc0, pb, 0)
                    tail_c(xt1, bc1, pb, 1)
                    return
                # final block: skip the staging copy (nothing reuses the xa
                # ring) — recip and muls read the psum accumulator directly;
                # all muls are emitted before the merge copies so DVE's FIFO
                # never parks a mul behind one; merge copies alternate
                # DVE/ACT (ACT is idle after the last exp).
                nh, hp = PBS[pb]
                nbase = nh * NHALF
                recs, xts = [], []
                recs_f32 = []
                dens, rawrecs = [], []
                lxps = [xps.pop((pb, 0)), xps.pop((pb, 1))]
                for lane in (0, 1):
                    den = sm.tile([1, NHALF], F32, tag=f"den{lane}", bufs=1, name=f"dnT{lane}")
                    nc.vector.tensor_copy(den, lxps[lane][HD : HD + 1, :])
                    dens.append(den)
                for lane in (0, 1):
                    rec = sm.tile([1, NHALF], F32, tag=f"rec{lane}", bufs=1, name=f"rcT{lane}")
                    nc.vector.reciprocal_approx_fast(out=rec, in_=dens[lane])
                    rawrecs.append(rec)
                for lane in (0, 1):
                    # ACT is idle after the last exp: staging copy + the
                    # fp32r-rounded rec copy both run there.
                    xt = sm.tile([HD, NHALF], F32, tag=f"xt{lane}", bufs=1, name=f"xtT{lane}")
                    nc.scalar.copy(xt, lxps[lane][0:HD, :])
                    xts.append(xt)
                for lane in (0, 1):
                    recR = sm.tile([1, NHALF], F32R, tag=f"recT{lane}", bufs=1, name=f"rcR{lane}")
                    nc.scalar.copy(recR, rawrecs[lane])
                    recs.append(recR)
                    recs_f32.append(rawrecs[lane])
                dbg_xt.append(xts[0])
                dbg_rec.append(recs_f32[0])
                bcs = [tail_b_pe(recs[lane], pb, lane) for lane in (0, 1)]
                for j in range(NHS):
                    js = slice(j * SW, (j + 1) * SW)
                    for lane in (0, 1):
                        rows = slice(lane * 64, lane * 64 + 64)
                        nc.vector.tensor_mul(
                            x_sb[hp][rows, nbase + j * SW : nbase + (j + 1) * SW],
                            xts[lane][:, js],
                            bcs[lane][0:64, js],
                        )
                for j in range(NHS):
                    for ot in range(KT):
                        merge_unit(ot, nh, j, act_copy=(ot % 2 == 1))

            # hook schedule: global step -> PE work units
            hooks = defaultdict(list)
            # PB0 (nh0, hp0): finish k ct0, produce all vt, q ct1 for PB1,
            # k ct1 for PB1.
            hooks[0].append(partial(vt_proj, 2))
            hooks[0].append(partial(k_proj, 1, 0))
            hooks[1].append(partial(vt_proj, 3))
            hooks[1].append(partial(q_proj, 0, 1))
            hooks[2].append(partial(vt_proj, 4))
            hooks[2].append(partial(q_proj, 1, 1))
            hooks[3].append(partial(vt_proj, 5))
            hooks[3].append(partial(k_proj, 2, 0))
            hooks[4].append(partial(vt_proj, 6))
            hooks[4].append(partial(k_proj, 0, 1))
            hooks[5].append(partial(vt_proj, 7))
            hooks[5].append(partial(k_proj, 1, 1))
            hooks[6].append(partial(vt_proj, 8))
            hooks[6].append(partial(k_proj, 3, 0))
            hooks[7].append(partial(vt_proj, 9))
            hooks[7].append(partial(k_proj, 2, 1))
            hooks[8].append(partial(vt_proj, 10))
            hooks[8].append(partial(k_proj, 3, 1))
            for i, mt in enumerate(range(11, MT)):
                hooks[9 + i].append(partial(vt_proj, mt))
            # PB1 (nh0, hp1): q slices 2,3 for nh=1
            hooks[MT + 0].append(partial(q_proj, 2, 0))
            hooks[MT + 1].append(partial(q_proj, 3, 0))
            hooks[MT + 2].append(partial(q_proj, 2, 1))
            hooks[MT + 3].append(partial(q_proj, 3, 1))
            # PB2 (nh1, hp0): merge nh=0, one 512-col unit per slot
            for i in range(KT * NHS):
                hooks[2 * MT + 2 * i + 1].append(partial(merge_unit, i // NHS, 0, i % NHS))

            # ---- prelude ---------------------------------------------------
            q_proj(0, 0)
            q_proj(1, 0)
            k_proj(0, 0, 0, 256)
            k_proj(0, 0, 256, SW)
            qk(0, 0)
            qk(0, 1)
            vt_proj(0)
            vt_proj(1)

            # ---- main loop -------------------------------------------------
            for g in range(GTOT):
                pb, mt = divmod(g, MT)
                prs = []
                for lane in (0, 1):
                    pr = sm.tile(
                        [128, NHALF], BF16, tag=f"pr{(2 * g + lane) % 6}",
                        name=f"pr{g}_{lane}",
                    )
                    nc.scalar.activation(pr, sdict.pop((g, lane)), EXP, scale=0.125)
                    prs.append(pr)
                if g + 1 < GTOT:
                    qk(g + 1, 0)
                    qk(g + 1, 1)
                pv(g, 0, prs[0])
                pv(g, 1, prs[1])
                if mt == MT - 1:
                    block_tail(pb)
                for fn in hooks.get(g, []):
                    fn()

            # (nh=1 merges are emitted inside the final block_tail)
            if debug:
                nc.sync.dma_start(dbg["dbg_q0"], q_sb[0])
                nc.sync.dma_start(dbg["dbg_k0"], k_sb[0])
                nc.sync.dma_start(dbg["dbg_vt0"], vt_sb[0])
                nc.sync.dma_start(dbg["dbg_x0"], x_sb[0])
                nc.sync.dma_start(dbg["dbg_x1"], x_sb[1])
                nc.sync.dma_start(dbg["dbg_xp"], dbg_xt[0])
                nc.sync.dma_start(dbg["dbg_rec"], dbg_rec[0])

        for rep in range(reps):
            emit_body(rep)

    nc.compile()
    return nc


def make_in_maps(query, key, value, Wq, bq, Wk, Wv, Wm, n_cores=NCORES):
    import ml_dtypes

    bf = ml_dtypes.bfloat16
    query = np.asarray(query, np.float32)
    key = np.asarray(key, np.float32)
    value = np.asarray(value, np.float32)
    Wq = np.asarray(Wq, np.float32)
    bq = np.asarray(bq, np.float32)
    Wk = np.asarray(Wk, np.float32)
    Wv = np.asarray(Wv, np.float32)
    Wm = np.asarray(Wm, np.float32)
    in_maps = []
    for c in range(n_cores):
        b, hg = c // 2, c % 2
        heads = [hg * HPC + i for i in range(HPC)]
        mych = np.array([d * H + h for h in heads for d in range(HD)])
        in_maps.append(
            {
                "xq": np.ascontiguousarray(query[b]).astype(bf),
                "xk": np.ascontiguousarray(key[b]).astype(bf),
                "xv": np.ascontiguousarray(value[b]).astype(bf),
                "wqT": np.ascontiguousarray(Wq[mych].T.astype(bf)),
                "wkT": np.ascontiguousarray(Wk[mych].T.astype(bf)),
                "wvT": np.ascontiguousarray(Wv[mych].T.astype(bf)),
                "wmT": np.ascontiguousarray(Wm[:, mych].T.astype(bf)),
                "bq2": np.ascontiguousarray(bq[mych].reshape(CT, 128).T),
                "onesc": np.ones((128, HPC), bf),
            }
        )
    return in_maps


_PROG = {}


def _get_program(N=N_FULL):
    if N not in _PROG:
        _PROG[N] = build_program(N)
    return _PROG[N]


def kernel(query, key, value, Wq, bq, Wk, bk, Wv, bv, Wm, bm):
    nc = _get_program()
    in_maps = make_in_maps(query, key, value, Wq, bq, Wk, Wv, Wm)
    res = run_bass_kernel_spmd(nc, in_maps, list(range(NCORES))).results
    bm_eff = (np.asarray(Wm, np.float64) @ np.asarray(bv, np.float64)).astype(
        np.float32
    ) + np.asarray(bm, np.float32)
    out = np.empty((B, D, N_FULL), np.float32)
    for b in range(B):
        out[b] = res[2 * b]["out"] + res[2 * b + 1]["out"] + bm_eff[:, None]
    return out
